# revision 10
# baseline (speedup 1.0000x reference)
"""RNN-T Joiner kernel for Trainium2, SPMD over 8 NeuronCores.

Reference computation (per batch b):
    hf = ft[b] @ w1[:, :ENC].T            # [T, J]
    hg = gu[b] @ w1[:, ENC:].T            # [U, J]
    joint = tanh(hf[:, None, :] + hg[None, :, :])   # [T, U, J]
    out[b] = joint @ w2.T                 # [T, U, V]

Sharding: data-parallel over B — each of the 8 cores handles one batch
element, full weights replicated. No collectives.

The kernel is output-write-bound: each core writes T*U*V*4B = 32.8 MB,
which saturates all 16 DMA queues for ~76 us. Everything else is
pipelined to keep that write stream dense:
  - inputs are loaded in small chunks (separate dma_starts) so PE
    transposes start as soon as each chunk lands;
  - transposes contract only over valid partitions (no memzero needed
    for the 125/64-row tails);
  - joint and w2 are bf16 for the big GEMM (full PE rate, ~1e-3 rel err);
  - the u-loop emits small output blocks first (UB ramp 1,1,2,4,8...)
    so the first output DMA is enqueued as early as possible.
"""

import numpy as np

import concourse.bass as bass
import concourse.mybir as mybir
import concourse.tile as tile
from concourse import bacc
from concourse.bass_utils import run_bass_kernel_spmd
from concourse.masks import make_identity

B, T, U = 8, 256, 64
ENC, PRED = 128, 256
J, V = 256, 500
N_CORES = 8
P = 128
NVO = 4          # w2 row chunks of 125
f32 = mybir.dt.float32
bf16 = mybir.dt.bfloat16

JO = J // P      # 2 chunks of j
TO = T // P      # 2 chunks of t
UBS = [8] * 8   # u-block sizes; sums to U


def _emit(nc, tc, ft, gu, w1, w2, out):
    assert sum(UBS) == U
    with (
        tc.tile_pool(name="const", bufs=1) as const,
        tc.tile_pool(name="jp", bufs=6) as jpool,
        tc.tile_pool(name="op", bufs=1) as opool,
    ):
        ident = const.tile([P, P], f32, tag="ident")
        make_identity(nc, ident)

        # ---- chunked natural-layout loads (separate tiles => precise deps) ----
        # dma_start issue (DIRECT2D descriptor generation, ~650ns each) is
        # spread across otherwise-idle engine queues so the 9 input rings
        # kick off in parallel instead of serializing on the sync engine.
        gu_sb = const.tile([P, PRED], f32, tag="gu")
        nc.sync.dma_start(gu_sb[:U, :], gu.ap())

        ft_sb = [const.tile([P, ENC], f32, tag=f"ft{to}", name=f"ft{to}") for to in range(TO)]
        for to in range(TO):
            nc.scalar.dma_start(ft_sb[to][:], ft.ap()[to * P : (to + 1) * P, :])
        # (only sync/scalar/gpsimd queues can initiate DMAs)

        w1_sb = [const.tile([P, ENC + PRED], f32, tag=f"w1_{jo}", name=f"w1_{jo}") for jo in range(JO)]
        for jo in range(JO):
            nc.gpsimd.dma_start(w1_sb[jo][:], w1.ap()[jo * P : (jo + 1) * P, :])

        w2_sb = [const.tile([P, J], f32, tag=f"w2_{vo}", name=f"w2_{vo}") for vo in range(NVO)]
        for vo in range(NVO):
            eng = nc.scalar if vo < 2 else nc.sync
            eng.dma_start(w2_sb[vo][:125, :], w2.ap()[vo * 125 : (vo + 1) * 125, :])

        # ---- prologue: PE transposes + small GEMMs, in data-arrival order ----
        pst_cm = tc.tile_pool(name="pst", bufs=2, space="PSUM")
        pst = pst_cm.__enter__()

        # guT[p, pc, u] (contract over the U=64 valid partitions only)
        guT = const.tile([P, PRED // P, U], f32, tag="guT")
        for pc in range(PRED // P):
            pt = pst.tile([P, U], f32, tag="ptq")
            nc.tensor.transpose(
                pt[:], gu_sb[:U, pc * P : (pc + 1) * P], ident[:U, :U]
            )
            nc.vector.tensor_copy(guT[:, pc, :], pt[:])

        # ftT[e, t]
        ftT = const.tile([P, T], f32, tag="ftT")
        for to in range(TO):
            pt = pst.tile([P, P], f32, tag="pt")
            nc.tensor.transpose(pt[:], ft_sb[to][:], ident[:])
            nc.vector.tensor_copy(ftT[:, to * P : (to + 1) * P], pt[:])

        # w1T for the encoder block: w1T0[e, j]
        w1T0 = const.tile([P, J], f32, tag="w1T0")
        for jo in range(JO):
            pt = pst.tile([P, P], f32, tag="pt")
            nc.tensor.transpose(pt[:], w1_sb[jo][:, :P], ident[:])
            nc.vector.tensor_copy(w1T0[:, jo * P : (jo + 1) * P], pt[:])

        # hfT[j, t] per jo chunk
        hfT = [const.tile([P, T], f32, tag=f"hfT{jo}", name=f"hfT{jo}") for jo in range(JO)]
        for jo in range(JO):
            ph = pst.tile([P, T], f32, tag="ph")
            nc.tensor.matmul(
                ph[:], w1T0[:, jo * P : (jo + 1) * P], ftT[:], start=True, stop=True
            )
            nc.vector.tensor_copy(hfT[jo][:], ph[:])

        # w1T for the predictor block: w1Tg[p, pc, j]
        w1Tg = const.tile([P, PRED // P, J], f32, tag="w1Tg")
        for pc in range(PRED // P):
            for jo in range(JO):
                pt = pst.tile([P, P], f32, tag="pt")
                nc.tensor.transpose(
                    pt[:], w1_sb[jo][:, (1 + pc) * P : (2 + pc) * P], ident[:]
                )
                nc.vector.tensor_copy(w1Tg[:, pc, jo * P : (jo + 1) * P], pt[:])

        # hgT[j, u] per jo chunk
        hgT = [const.tile([P, U], f32, tag=f"hgT{jo}", name=f"hgT{jo}") for jo in range(JO)]
        for jo in range(JO):
            ph = pst.tile([P, U], f32, tag="phg")
            for pc in range(PRED // P):
                nc.tensor.matmul(
                    ph[:],
                    w1Tg[:, pc, jo * P : (jo + 1) * P],
                    guT[:, pc, :],
                    start=(pc == 0),
                    stop=(pc == PRED // P - 1),
                )
            nc.vector.tensor_copy(hgT[jo][:], ph[:])

        # w2T[j, jo, v] in bf16 (contract over the 125 valid partitions only)
        w2T = const.tile([P, JO, V], bf16, tag="w2T")
        for vo in range(NVO):
            for jo in range(JO):
                pt = pst.tile([P, P], f32, tag="pt")
                nc.tensor.transpose(
                    pt[:, :125],
                    w2_sb[vo][:125, jo * P : (jo + 1) * P],
                    ident[:125, :125],
                )
                nc.vector.tensor_copy(
                    w2T[:, jo, vo * 125 : (vo + 1) * 125], pt[:, :125]
                )

        pst_cm.__exit__(None, None, None)

        # ---- main loop over u blocks ----
        pso_cm = tc.tile_pool(name="pso", bufs=6, space="PSUM")
        pso = pso_cm.__enter__()
        u0 = 0
        for bi, UB in enumerate(UBS):
            steady = UB == 8
            ot = [
                opool.tile(
                    [P, UB, V],
                    f32,
                    tag=(f"ot{to}_s" if steady else f"ot{to}_r{bi}"),
                    name=f"ot{to}_b{bi}",
                    bufs=(4 if steady else 1),
                )
                for to in range(TO)
            ]
            for uo in range(UB):
                u = u0 + uo
                joint = []
                for jo in range(JO):
                    jt = jpool.tile([P, T], bf16, tag="joint")
                    nc.scalar.activation(
                        jt[:],
                        hfT[jo][:],
                        mybir.ActivationFunctionType.Tanh,
                        bias=hgT[jo][:, u : u + 1],
                        scale=1.0,
                    )
                    joint.append(jt)
                for to in range(TO):
                    po = pso.tile([P, V], f32, tag="po")
                    for jo in range(JO):
                        nc.tensor.matmul(
                            po[:],
                            joint[jo][:, to * P : (to + 1) * P],
                            w2T[:, jo, :],
                            start=(jo == 0),
                            stop=(jo == JO - 1),
                        )
                    # split PSUM->SBUF copies across two engines: they are
                    # the steady-state pacing op at ~600ns per [128,500] tile
                    if to == 0:
                        nc.vector.tensor_copy(ot[to][:, uo, :], po[:])
                    else:
                        nc.scalar.copy(ot[to][:, uo, :], po[:])
            for to in range(TO):
                nc.sync.dma_start(
                    out.ap()[to * P : (to + 1) * P, u0 : u0 + UB, :], ot[to][:]
                )
            u0 += UB
        pso_cm.__exit__(None, None, None)


_NC_CACHE = None


def _build():
    global _NC_CACHE
    if _NC_CACHE is not None:
        return _NC_CACHE
    nc = bacc.Bacc("TRN2", target_bir_lowering=False, debug=False)
    ft = nc.dram_tensor("ft", [T, ENC], f32, kind="ExternalInput")
    gu = nc.dram_tensor("gu", [U, PRED], f32, kind="ExternalInput")
    w1 = nc.dram_tensor("w1", [J, ENC + PRED], f32, kind="ExternalInput")
    w2 = nc.dram_tensor("w2", [V, J], f32, kind="ExternalInput")
    out = nc.dram_tensor("out", [T, U, V], f32, kind="ExternalOutput")
    with tile.TileContext(nc) as tc:
        _emit(nc, tc, ft, gu, w1, w2, out)
    nc.compile()
    _NC_CACHE = nc
    return nc


def run(ft, gu, w1, w2, trace=False):
    """Run the SPMD kernel; returns (output [B,T,U,V], BassKernelResults)."""
    nc = _build()
    w1c = np.ascontiguousarray(w1, dtype=np.float32)
    w2c = np.ascontiguousarray(w2, dtype=np.float32)
    in_maps = [
        {
            "ft": np.ascontiguousarray(ft[b], dtype=np.float32),
            "gu": np.ascontiguousarray(gu[b], dtype=np.float32),
            "w1": w1c,
            "w2": w2c,
        }
        for b in range(B)
    ]
    res = run_bass_kernel_spmd(
        nc, in_maps, core_ids=list(range(N_CORES)), trace=trace
    )
    out = np.stack([res.results[c]["out"] for c in range(N_CORES)], axis=0)
    return out, res


def kernel(ft, gu, w1, w2):
    out, _ = run(ft, gu, w1, w2, trace=False)
    return out


# revision 11
# speedup vs baseline: 1.1167x; 1.1167x over previous
"""RNN-T Joiner kernel for Trainium2, SPMD over 8 NeuronCores.

Reference computation (per batch b):
    hf = ft[b] @ w1[:, :ENC].T            # [T, J]
    hg = gu[b] @ w1[:, ENC:].T            # [U, J]
    joint = tanh(hf[:, None, :] + hg[None, :, :])   # [T, U, J]
    out[b] = joint @ w2.T                 # [T, U, V]

Sharding: data-parallel over B — each of the 8 cores handles one batch
element, full weights replicated. No collectives.

The kernel is output-write-bound: each core writes T*U*V*4B = 32.8 MB,
which saturates all 16 DMA queues for ~76 us. Everything else is
pipelined to keep that write stream dense:
  - inputs are loaded in small chunks (separate dma_starts) so PE
    transposes start as soon as each chunk lands;
  - transposes contract only over valid partitions (no memzero needed
    for the 125/64-row tails);
  - joint and w2 are bf16 for the big GEMM (full PE rate, ~1e-3 rel err);
  - the u-loop emits small output blocks first (UB ramp 1,1,2,4,8...)
    so the first output DMA is enqueued as early as possible.
"""

import numpy as np

import concourse.bass as bass
import concourse.mybir as mybir
import concourse.tile as tile
from concourse import bacc
from concourse.bass_utils import run_bass_kernel_spmd
from concourse.masks import make_identity

B, T, U = 8, 256, 64
ENC, PRED = 128, 256
J, V = 256, 500
N_CORES = 8
P = 128
NVO = 4          # w2 row chunks of 125
f32 = mybir.dt.float32
bf16 = mybir.dt.bfloat16

JO = J // P      # 2 chunks of j
TO = T // P      # 2 chunks of t
UBS = [8] * 8   # u-block sizes; sums to U


def _emit(nc, tc, ft, gu, w1, w2, out):
    assert sum(UBS) == U
    with (
        tc.tile_pool(name="const", bufs=1) as const,
        tc.tile_pool(name="jp", bufs=6) as jpool,
        tc.tile_pool(name="op", bufs=1) as opool,
    ):
        ident = const.tile([P, P], f32, tag="ident")
        make_identity(nc, ident)

        # ---- chunked natural-layout loads (separate tiles => precise deps) ----
        # dma_start issue (DIRECT2D descriptor generation, ~650ns each) is
        # spread across otherwise-idle engine queues so the 9 input rings
        # kick off in parallel instead of serializing on the sync engine.
        gu_sb = const.tile([P, PRED], f32, tag="gu")
        nc.sync.dma_start(gu_sb[:U, :], gu.ap())

        ft_sb = [const.tile([P, ENC], f32, tag=f"ft{to}", name=f"ft{to}") for to in range(TO)]
        for to in range(TO):
            nc.scalar.dma_start(ft_sb[to][:], ft.ap()[to * P : (to + 1) * P, :])
        # (only sync/scalar/gpsimd queues can initiate DMAs)

        w1_sb = [const.tile([P, ENC + PRED], f32, tag=f"w1_{jo}", name=f"w1_{jo}") for jo in range(JO)]
        for jo in range(JO):
            nc.gpsimd.dma_start(w1_sb[jo][:], w1.ap()[jo * P : (jo + 1) * P, :])

        w2_sb = [const.tile([P, J], f32, tag=f"w2_{vo}", name=f"w2_{vo}") for vo in range(NVO)]
        for vo in range(NVO):
            eng = nc.scalar if vo < 2 else nc.sync
            eng.dma_start(w2_sb[vo][:125, :], w2.ap()[vo * 125 : (vo + 1) * 125, :])

        # ---- prologue: PE transposes + small GEMMs, in data-arrival order ----
        pst_cm = tc.tile_pool(name="pst", bufs=2, space="PSUM")
        pst = pst_cm.__enter__()

        # guT[p, pc, u] (contract over the U=64 valid partitions only)
        guT = const.tile([P, PRED // P, U], f32, tag="guT")
        for pc in range(PRED // P):
            pt = pst.tile([P, U], f32, tag="ptq")
            nc.tensor.transpose(
                pt[:], gu_sb[:U, pc * P : (pc + 1) * P], ident[:U, :U]
            )
            nc.vector.tensor_copy(guT[:, pc, :], pt[:])

        # ftT[e, t]
        ftT = const.tile([P, T], f32, tag="ftT")
        for to in range(TO):
            pt = pst.tile([P, P], f32, tag="pt")
            nc.tensor.transpose(pt[:], ft_sb[to][:], ident[:])
            nc.vector.tensor_copy(ftT[:, to * P : (to + 1) * P], pt[:])

        # w1T for the encoder block: w1T0[e, j]
        w1T0 = const.tile([P, J], f32, tag="w1T0")
        for jo in range(JO):
            pt = pst.tile([P, P], f32, tag="pt")
            nc.tensor.transpose(pt[:], w1_sb[jo][:, :P], ident[:])
            nc.vector.tensor_copy(w1T0[:, jo * P : (jo + 1) * P], pt[:])

        # hfT[j, t] per jo chunk
        hfT = [const.tile([P, T], f32, tag=f"hfT{jo}", name=f"hfT{jo}") for jo in range(JO)]
        for jo in range(JO):
            ph = pst.tile([P, T], f32, tag="ph")
            nc.tensor.matmul(
                ph[:], w1T0[:, jo * P : (jo + 1) * P], ftT[:], start=True, stop=True
            )
            nc.vector.tensor_copy(hfT[jo][:], ph[:])

        # w1T for the predictor block: w1Tg[p, pc, j]
        w1Tg = const.tile([P, PRED // P, J], f32, tag="w1Tg")
        for pc in range(PRED // P):
            for jo in range(JO):
                pt = pst.tile([P, P], f32, tag="pt")
                nc.tensor.transpose(
                    pt[:], w1_sb[jo][:, (1 + pc) * P : (2 + pc) * P], ident[:]
                )
                nc.vector.tensor_copy(w1Tg[:, pc, jo * P : (jo + 1) * P], pt[:])

        # hgT[j, u] per jo chunk
        hgT = [const.tile([P, U], f32, tag=f"hgT{jo}", name=f"hgT{jo}") for jo in range(JO)]
        for jo in range(JO):
            ph = pst.tile([P, U], f32, tag="phg")
            for pc in range(PRED // P):
                nc.tensor.matmul(
                    ph[:],
                    w1Tg[:, pc, jo * P : (jo + 1) * P],
                    guT[:, pc, :],
                    start=(pc == 0),
                    stop=(pc == PRED // P - 1),
                )
            nc.vector.tensor_copy(hgT[jo][:], ph[:])

        # w2T[j, jo, v] in bf16 (contract over the 125 valid partitions only)
        w2T = const.tile([P, JO, V], bf16, tag="w2T")
        for vo in range(NVO):
            for jo in range(JO):
                pt = pst.tile([P, P], f32, tag="pt")
                nc.tensor.transpose(
                    pt[:, :125],
                    w2_sb[vo][:125, jo * P : (jo + 1) * P],
                    ident[:125, :125],
                )
                nc.vector.tensor_copy(
                    w2T[:, jo, vo * 125 : (vo + 1) * 125], pt[:, :125]
                )

        pst_cm.__exit__(None, None, None)

        # ---- main loop over u blocks ----
        pso_cm = tc.tile_pool(name="pso", bufs=6, space="PSUM")
        pso = pso_cm.__enter__()
        u0 = 0
        for bi, UB in enumerate(UBS):
            steady = UB == 8
            ot = [
                opool.tile(
                    [P, UB, V],
                    f32,
                    tag=(f"ot{to}_s" if steady else f"ot{to}_r{bi}"),
                    name=f"ot{to}_b{bi}",
                    bufs=(4 if steady else 1),
                )
                for to in range(TO)
            ]
            for uo in range(UB):
                u = u0 + uo
                joint = []
                for jo in range(JO):
                    jt = jpool.tile([P, T], bf16, tag="joint")
                    nc.scalar.activation(
                        jt[:],
                        hfT[jo][:],
                        mybir.ActivationFunctionType.Tanh,
                        bias=hgT[jo][:, u : u + 1],
                        scale=1.0,
                    )
                    joint.append(jt)
                for to in range(TO):
                    po = pso.tile([P, V], f32, tag="po")
                    for jo in range(JO):
                        nc.tensor.matmul(
                            po[:],
                            joint[jo][:, to * P : (to + 1) * P],
                            w2T[:, jo, :],
                            start=(jo == 0),
                            stop=(jo == JO - 1),
                        )
                    # split PSUM->SBUF copies across two engines: they are
                    # the steady-state pacing op at ~600ns per [128,500] tile
                    if to == 0 or u % 2 == 0:
                        nc.vector.tensor_copy(ot[to][:, uo, :], po[:])
                    else:
                        nc.scalar.copy(ot[to][:, uo, :], po[:])
            for to in range(TO):
                nc.sync.dma_start(
                    out.ap()[to * P : (to + 1) * P, u0 : u0 + UB, :], ot[to][:]
                )
            u0 += UB
        pso_cm.__exit__(None, None, None)


_NC_CACHE = None


def _build():
    global _NC_CACHE
    if _NC_CACHE is not None:
        return _NC_CACHE
    nc = bacc.Bacc("TRN2", target_bir_lowering=False, debug=False)
    ft = nc.dram_tensor("ft", [T, ENC], f32, kind="ExternalInput")
    gu = nc.dram_tensor("gu", [U, PRED], f32, kind="ExternalInput")
    w1 = nc.dram_tensor("w1", [J, ENC + PRED], f32, kind="ExternalInput")
    w2 = nc.dram_tensor("w2", [V, J], f32, kind="ExternalInput")
    out = nc.dram_tensor("out", [T, U, V], f32, kind="ExternalOutput")
    with tile.TileContext(nc) as tc:
        _emit(nc, tc, ft, gu, w1, w2, out)
    nc.compile()
    _NC_CACHE = nc
    return nc


def run(ft, gu, w1, w2, trace=False):
    """Run the SPMD kernel; returns (output [B,T,U,V], BassKernelResults)."""
    nc = _build()
    w1c = np.ascontiguousarray(w1, dtype=np.float32)
    w2c = np.ascontiguousarray(w2, dtype=np.float32)
    in_maps = [
        {
            "ft": np.ascontiguousarray(ft[b], dtype=np.float32),
            "gu": np.ascontiguousarray(gu[b], dtype=np.float32),
            "w1": w1c,
            "w2": w2c,
        }
        for b in range(B)
    ]
    res = run_bass_kernel_spmd(
        nc, in_maps, core_ids=list(range(N_CORES)), trace=trace
    )
    out = np.stack([res.results[c]["out"] for c in range(N_CORES)], axis=0)
    return out, res


def kernel(ft, gu, w1, w2):
    out, _ = run(ft, gu, w1, w2, trace=False)
    return out


# revision 13
# speedup vs baseline: 1.1290x; 1.0110x over previous
"""RNN-T Joiner kernel for Trainium2, SPMD over 8 NeuronCores.

Reference computation (per batch b):
    hf = ft[b] @ w1[:, :ENC].T            # [T, J]
    hg = gu[b] @ w1[:, ENC:].T            # [U, J]
    joint = tanh(hf[:, None, :] + hg[None, :, :])   # [T, U, J]
    out[b] = joint @ w2.T                 # [T, U, V]

Sharding: data-parallel over B — each of the 8 cores handles one batch
element, full weights replicated. No collectives.

The kernel is output-write-bound: each core writes T*U*V*4B = 32.8 MB,
which saturates all 16 DMA queues for ~76 us. Everything else is
pipelined to keep that write stream dense:
  - inputs are loaded in small chunks (separate dma_starts) so PE
    transposes start as soon as each chunk lands;
  - transposes contract only over valid partitions (no memzero needed
    for the 125/64-row tails);
  - joint and w2 are bf16 for the big GEMM (full PE rate, ~1e-3 rel err);
  - the u-loop emits small output blocks first (UB ramp 1,1,2,4,8...)
    so the first output DMA is enqueued as early as possible.
"""

import numpy as np

import concourse.bass as bass
import concourse.mybir as mybir
import concourse.tile as tile
from concourse import bacc
from concourse.bass_utils import run_bass_kernel_spmd
from concourse.masks import make_identity

B, T, U = 8, 256, 64
ENC, PRED = 128, 256
J, V = 256, 500
N_CORES = 8
P = 128
NVO = 4          # w2 row chunks of 125
f32 = mybir.dt.float32
bf16 = mybir.dt.bfloat16

JO = J // P      # 2 chunks of j
TO = T // P      # 2 chunks of t
UBS = [2, 2, 4] + [8] * 7   # u-block ramp; sums to U


def _emit(nc, tc, ft, gu, w1, w2, out):
    assert sum(UBS) == U
    with (
        tc.tile_pool(name="const", bufs=1) as const,
        tc.tile_pool(name="jp", bufs=6) as jpool,
        tc.tile_pool(name="op", bufs=1) as opool,
    ):
        ident = const.tile([P, P], f32, tag="ident")
        make_identity(nc, ident)

        # ---- chunked natural-layout loads (separate tiles => precise deps) ----
        # dma_start issue (DIRECT2D descriptor generation, ~650ns each) is
        # spread across otherwise-idle engine queues so the 9 input rings
        # kick off in parallel instead of serializing on the sync engine.
        gu_sb = const.tile([P, PRED], f32, tag="gu")
        nc.sync.dma_start(gu_sb[:U, :], gu.ap())

        ft_sb = [const.tile([P, ENC], f32, tag=f"ft{to}", name=f"ft{to}") for to in range(TO)]
        for to in range(TO):
            nc.scalar.dma_start(ft_sb[to][:], ft.ap()[to * P : (to + 1) * P, :])
        # (only sync/scalar/gpsimd queues can initiate DMAs)

        w1_sb = [const.tile([P, ENC + PRED], f32, tag=f"w1_{jo}", name=f"w1_{jo}") for jo in range(JO)]
        for jo in range(JO):
            nc.gpsimd.dma_start(w1_sb[jo][:], w1.ap()[jo * P : (jo + 1) * P, :])

        w2_sb = [const.tile([P, J], f32, tag=f"w2_{vo}", name=f"w2_{vo}") for vo in range(NVO)]
        for vo in range(NVO):
            eng = nc.scalar if vo < 2 else nc.sync
            eng.dma_start(w2_sb[vo][:125, :], w2.ap()[vo * 125 : (vo + 1) * 125, :])

        # ---- prologue: PE transposes + small GEMMs, in data-arrival order ----
        pst_cm = tc.tile_pool(name="pst", bufs=2, space="PSUM")
        pst = pst_cm.__enter__()

        # guT[p, pc, u] (contract over the U=64 valid partitions only)
        guT = const.tile([P, PRED // P, U], f32, tag="guT")
        for pc in range(PRED // P):
            pt = pst.tile([P, U], f32, tag="ptq")
            nc.tensor.transpose(
                pt[:], gu_sb[:U, pc * P : (pc + 1) * P], ident[:U, :U]
            )
            nc.vector.tensor_copy(guT[:, pc, :], pt[:])

        # ftT[e, t]
        ftT = const.tile([P, T], f32, tag="ftT")
        for to in range(TO):
            pt = pst.tile([P, P], f32, tag="pt")
            nc.tensor.transpose(pt[:], ft_sb[to][:], ident[:])
            nc.vector.tensor_copy(ftT[:, to * P : (to + 1) * P], pt[:])

        # w1T for the encoder block: w1T0[e, j]
        w1T0 = const.tile([P, J], f32, tag="w1T0")
        for jo in range(JO):
            pt = pst.tile([P, P], f32, tag="pt")
            nc.tensor.transpose(pt[:], w1_sb[jo][:, :P], ident[:])
            nc.vector.tensor_copy(w1T0[:, jo * P : (jo + 1) * P], pt[:])

        # hfT[j, t] per jo chunk
        hfT = [const.tile([P, T], f32, tag=f"hfT{jo}", name=f"hfT{jo}") for jo in range(JO)]
        for jo in range(JO):
            ph = pst.tile([P, T], f32, tag="ph")
            nc.tensor.matmul(
                ph[:], w1T0[:, jo * P : (jo + 1) * P], ftT[:], start=True, stop=True
            )
            nc.vector.tensor_copy(hfT[jo][:], ph[:])

        # w1T for the predictor block: w1Tg[p, pc, j]
        w1Tg = const.tile([P, PRED // P, J], f32, tag="w1Tg")
        for pc in range(PRED // P):
            for jo in range(JO):
                pt = pst.tile([P, P], f32, tag="pt")
                nc.tensor.transpose(
                    pt[:], w1_sb[jo][:, (1 + pc) * P : (2 + pc) * P], ident[:]
                )
                nc.vector.tensor_copy(w1Tg[:, pc, jo * P : (jo + 1) * P], pt[:])

        # hgT[j, u] per jo chunk
        hgT = [const.tile([P, U], f32, tag=f"hgT{jo}", name=f"hgT{jo}") for jo in range(JO)]
        for jo in range(JO):
            ph = pst.tile([P, U], f32, tag="phg")
            for pc in range(PRED // P):
                nc.tensor.matmul(
                    ph[:],
                    w1Tg[:, pc, jo * P : (jo + 1) * P],
                    guT[:, pc, :],
                    start=(pc == 0),
                    stop=(pc == PRED // P - 1),
                )
            nc.vector.tensor_copy(hgT[jo][:], ph[:])

        # w2T[j, jo, v] in bf16 (contract over the 125 valid partitions only)
        w2T = const.tile([P, JO, V], bf16, tag="w2T")
        for vo in range(NVO):
            for jo in range(JO):
                pt = pst.tile([P, P], f32, tag="pt")
                nc.tensor.transpose(
                    pt[:, :125],
                    w2_sb[vo][:125, jo * P : (jo + 1) * P],
                    ident[:125, :125],
                )
                nc.vector.tensor_copy(
                    w2T[:, jo, vo * 125 : (vo + 1) * 125], pt[:, :125]
                )

        pst_cm.__exit__(None, None, None)

        # ---- main loop over u blocks ----
        # output viewed as [p, to, u, v]: partition p carries DRAM rows
        # t = p and t = 128 + p, so one dma_start covers a whole u-block
        # (2 descriptors of UB*2000B per partition, single ring per block)
        outv = out.ap().rearrange("(to p) u v -> p to u v", p=P)
        pso_cm = tc.tile_pool(name="pso", bufs=6, space="PSUM")
        pso = pso_cm.__enter__()
        u0 = 0
        for bi, UB in enumerate(UBS):
            steady = UB == 8
            ot = opool.tile(
                [P, TO, UB, V],
                f32,
                tag=("ot_s" if steady else f"ot_r{bi}"),
                name=f"ot_b{bi}",
                bufs=(4 if steady else 1),
            )
            for uo in range(UB):
                u = u0 + uo
                joint = []
                for jo in range(JO):
                    jt = jpool.tile([P, T], bf16, tag="joint")
                    nc.scalar.activation(
                        jt[:],
                        hfT[jo][:],
                        mybir.ActivationFunctionType.Tanh,
                        bias=hgT[jo][:, u : u + 1],
                        scale=1.0,
                    )
                    joint.append(jt)
                for to in range(TO):
                    po = pso.tile([P, V], f32, tag="po")
                    for jo in range(JO):
                        nc.tensor.matmul(
                            po[:],
                            joint[jo][:, to * P : (to + 1) * P],
                            w2T[:, jo, :],
                            start=(jo == 0),
                            stop=(jo == JO - 1),
                        )
                    # split PSUM->SBUF copies across two engines: they are
                    # the steady-state pacing op at ~600ns per [128,500] tile
                    if to == 0 or u % 2 == 0:
                        nc.vector.tensor_copy(ot[:, to, uo, :], po[:])
                    else:
                        nc.scalar.copy(ot[:, to, uo, :], po[:])
            nc.sync.dma_start(outv[:, :, u0 : u0 + UB, :], ot[:])
            u0 += UB
        pso_cm.__exit__(None, None, None)


_NC_CACHE = None


def _build():
    global _NC_CACHE
    if _NC_CACHE is not None:
        return _NC_CACHE
    nc = bacc.Bacc("TRN2", target_bir_lowering=False, debug=False)
    ft = nc.dram_tensor("ft", [T, ENC], f32, kind="ExternalInput")
    gu = nc.dram_tensor("gu", [U, PRED], f32, kind="ExternalInput")
    w1 = nc.dram_tensor("w1", [J, ENC + PRED], f32, kind="ExternalInput")
    w2 = nc.dram_tensor("w2", [V, J], f32, kind="ExternalInput")
    out = nc.dram_tensor("out", [T, U, V], f32, kind="ExternalOutput")
    with tile.TileContext(nc) as tc:
        _emit(nc, tc, ft, gu, w1, w2, out)
    nc.compile()
    _NC_CACHE = nc
    return nc


def run(ft, gu, w1, w2, trace=False):
    """Run the SPMD kernel; returns (output [B,T,U,V], BassKernelResults)."""
    nc = _build()
    w1c = np.ascontiguousarray(w1, dtype=np.float32)
    w2c = np.ascontiguousarray(w2, dtype=np.float32)
    in_maps = [
        {
            "ft": np.ascontiguousarray(ft[b], dtype=np.float32),
            "gu": np.ascontiguousarray(gu[b], dtype=np.float32),
            "w1": w1c,
            "w2": w2c,
        }
        for b in range(B)
    ]
    res = run_bass_kernel_spmd(
        nc, in_maps, core_ids=list(range(N_CORES)), trace=trace
    )
    out = np.stack([res.results[c]["out"] for c in range(N_CORES)], axis=0)
    return out, res


def kernel(ft, gu, w1, w2):
    out, _ = run(ft, gu, w1, w2, trace=False)
    return out


# revision 16
# speedup vs baseline: 1.1821x; 1.0470x over previous
"""RNN-T Joiner kernel for Trainium2, SPMD over 8 NeuronCores.

Reference computation (per batch b):
    hf = ft[b] @ w1[:, :ENC].T            # [T, J]
    hg = gu[b] @ w1[:, ENC:].T            # [U, J]
    joint = tanh(hf[:, None, :] + hg[None, :, :])   # [T, U, J]
    out[b] = joint @ w2.T                 # [T, U, V]

Sharding: data-parallel over B — each of the 8 cores handles one batch
element, full weights replicated. No collectives.

The kernel is output-write-bound: each core writes T*U*V*4B = 32.8 MB,
which saturates all 16 DMA queues for ~76 us. Everything else is
pipelined to keep that write stream dense:
  - inputs are loaded in small chunks (separate dma_starts) so PE
    transposes start as soon as each chunk lands;
  - transposes contract only over valid partitions (no memzero needed
    for the 125/64-row tails);
  - joint and w2 are bf16 for the big GEMM (full PE rate, ~1e-3 rel err);
  - the u-loop emits small output blocks first (UB ramp 1,1,2,4,8...)
    so the first output DMA is enqueued as early as possible.
"""

import numpy as np

import concourse.bass as bass
import concourse.mybir as mybir
import concourse.tile as tile
from concourse import bacc
from concourse.bass_utils import run_bass_kernel_spmd
from concourse.masks import make_identity

B, T, U = 8, 256, 64
ENC, PRED = 128, 256
J, V = 256, 500
N_CORES = 8
P = 128
NVO = 4          # w2 row chunks of 125
f32 = mybir.dt.float32
bf16 = mybir.dt.bfloat16

JO = J // P      # 2 chunks of j
TO = T // P      # 2 chunks of t
UBS = [2, 2] + [4] * 15   # u-block ramp; sums to U


def _emit(nc, tc, ft, gu, w1, w2, out):
    assert sum(UBS) == U
    with (
        tc.tile_pool(name="const", bufs=1) as const,
        tc.tile_pool(name="jp", bufs=6) as jpool,
        tc.tile_pool(name="op", bufs=1) as opool,
    ):
        ident = const.tile([P, P], f32, tag="ident")
        make_identity(nc, ident)

        # ---- chunked natural-layout loads (separate tiles => precise deps) ----
        # dma_start issue (DIRECT2D descriptor generation, ~650ns each) is
        # spread across otherwise-idle engine queues so the 9 input rings
        # kick off in parallel instead of serializing on the sync engine.
        gu_sb = const.tile([P, PRED], f32, tag="gu")
        nc.sync.dma_start(gu_sb[:U, :], gu.ap())

        ft_sb = [const.tile([P, ENC], f32, tag=f"ft{to}", name=f"ft{to}") for to in range(TO)]
        for to in range(TO):
            nc.scalar.dma_start(ft_sb[to][:], ft.ap()[to * P : (to + 1) * P, :])
        # (only sync/scalar/gpsimd queues can initiate DMAs)

        w1_sb = [const.tile([P, ENC + PRED], f32, tag=f"w1_{jo}", name=f"w1_{jo}") for jo in range(JO)]
        for jo in range(JO):
            nc.gpsimd.dma_start(w1_sb[jo][:], w1.ap()[jo * P : (jo + 1) * P, :])

        w2_sb = [const.tile([P, J], f32, tag=f"w2_{vo}", name=f"w2_{vo}") for vo in range(NVO)]
        for vo in range(NVO):
            eng = nc.scalar if vo < 2 else nc.sync
            eng.dma_start(w2_sb[vo][:125, :], w2.ap()[vo * 125 : (vo + 1) * 125, :])

        # ---- prologue: PE transposes + small GEMMs, in data-arrival order ----
        pst_cm = tc.tile_pool(name="pst", bufs=2, space="PSUM")
        pst = pst_cm.__enter__()

        # guT[p, pc, u] (contract over the U=64 valid partitions only)
        guT = const.tile([P, PRED // P, U], f32, tag="guT")
        for pc in range(PRED // P):
            pt = pst.tile([P, U], f32, tag="ptq")
            nc.tensor.transpose(
                pt[:], gu_sb[:U, pc * P : (pc + 1) * P], ident[:U, :U]
            )
            nc.vector.tensor_copy(guT[:, pc, :], pt[:])

        # ftT[e, t]
        ftT = const.tile([P, T], f32, tag="ftT")
        for to in range(TO):
            pt = pst.tile([P, P], f32, tag="pt")
            nc.tensor.transpose(pt[:], ft_sb[to][:], ident[:])
            nc.vector.tensor_copy(ftT[:, to * P : (to + 1) * P], pt[:])

        # w1T for the encoder block: w1T0[e, j]
        w1T0 = const.tile([P, J], f32, tag="w1T0")
        for jo in range(JO):
            pt = pst.tile([P, P], f32, tag="pt")
            nc.tensor.transpose(pt[:], w1_sb[jo][:, :P], ident[:])
            nc.vector.tensor_copy(w1T0[:, jo * P : (jo + 1) * P], pt[:])

        # hfT[j, t] per jo chunk
        hfT = [const.tile([P, T], f32, tag=f"hfT{jo}", name=f"hfT{jo}") for jo in range(JO)]
        for jo in range(JO):
            ph = pst.tile([P, T], f32, tag="ph")
            nc.tensor.matmul(
                ph[:], w1T0[:, jo * P : (jo + 1) * P], ftT[:], start=True, stop=True
            )
            nc.vector.tensor_copy(hfT[jo][:], ph[:])

        # w1T for the predictor block: w1Tg[p, pc, j]
        w1Tg = const.tile([P, PRED // P, J], f32, tag="w1Tg")
        for pc in range(PRED // P):
            for jo in range(JO):
                pt = pst.tile([P, P], f32, tag="pt")
                nc.tensor.transpose(
                    pt[:], w1_sb[jo][:, (1 + pc) * P : (2 + pc) * P], ident[:]
                )
                nc.vector.tensor_copy(w1Tg[:, pc, jo * P : (jo + 1) * P], pt[:])

        # hgT[j, u] per jo chunk
        hgT = [const.tile([P, U], f32, tag=f"hgT{jo}", name=f"hgT{jo}") for jo in range(JO)]
        for jo in range(JO):
            ph = pst.tile([P, U], f32, tag="phg")
            for pc in range(PRED // P):
                nc.tensor.matmul(
                    ph[:],
                    w1Tg[:, pc, jo * P : (jo + 1) * P],
                    guT[:, pc, :],
                    start=(pc == 0),
                    stop=(pc == PRED // P - 1),
                )
            nc.vector.tensor_copy(hgT[jo][:], ph[:])

        # w2T[j, jo, v] in bf16 (contract over the 125 valid partitions only)
        w2T = const.tile([P, JO, V], bf16, tag="w2T")
        for vo in range(NVO):
            for jo in range(JO):
                pt = pst.tile([P, P], f32, tag="pt")
                nc.tensor.transpose(
                    pt[:, :125],
                    w2_sb[vo][:125, jo * P : (jo + 1) * P],
                    ident[:125, :125],
                )
                nc.vector.tensor_copy(
                    w2T[:, jo, vo * 125 : (vo + 1) * 125], pt[:, :125]
                )

        pst_cm.__exit__(None, None, None)

        # ---- main loop over u blocks ----
        # output viewed as [p, to, u, v]: partition p carries DRAM rows
        # t = p and t = 128 + p, so one dma_start covers a whole u-block
        # (2 descriptors of UB*2000B per partition, single ring per block)
        outv = out.ap().rearrange("(to p) u v -> p to u v", p=P)
        pso_cm = tc.tile_pool(name="pso", bufs=6, space="PSUM")
        pso = pso_cm.__enter__()
        u0 = 0
        for bi, UB in enumerate(UBS):
            steady = UB == 4
            ot = opool.tile(
                [P, TO, UB, V],
                f32,
                tag=("ot_s" if steady else f"ot_r{bi}"),
                name=f"ot_b{bi}",
                bufs=(6 if steady else 1),
            )
            for uo in range(UB):
                u = u0 + uo
                joint = []
                for jo in range(JO):
                    jt = jpool.tile([P, T], bf16, tag="joint")
                    nc.scalar.activation(
                        jt[:],
                        hfT[jo][:],
                        mybir.ActivationFunctionType.Tanh,
                        bias=hgT[jo][:, u : u + 1],
                        scale=1.0,
                    )
                    joint.append(jt)
                for to in range(TO):
                    po = pso.tile([P, V], f32, tag="po")
                    for jo in range(JO):
                        nc.tensor.matmul(
                            po[:],
                            joint[jo][:, to * P : (to + 1) * P],
                            w2T[:, jo, :],
                            start=(jo == 0),
                            stop=(jo == JO - 1),
                        )
                    # split PSUM->SBUF copies across two engines: they are
                    # the steady-state pacing op at ~600ns per [128,500] tile
                    if to == 0 or u % 2 == 0:
                        nc.vector.tensor_copy(ot[:, to, uo, :], po[:])
                    else:
                        nc.scalar.copy(ot[:, to, uo, :], po[:])
            nc.sync.dma_start(outv[:, :, u0 : u0 + UB, :], ot[:])
            u0 += UB
        pso_cm.__exit__(None, None, None)


_NC_CACHE = None


def _build():
    global _NC_CACHE
    if _NC_CACHE is not None:
        return _NC_CACHE
    nc = bacc.Bacc("TRN2", target_bir_lowering=False, debug=False)
    ft = nc.dram_tensor("ft", [T, ENC], f32, kind="ExternalInput")
    gu = nc.dram_tensor("gu", [U, PRED], f32, kind="ExternalInput")
    w1 = nc.dram_tensor("w1", [J, ENC + PRED], f32, kind="ExternalInput")
    w2 = nc.dram_tensor("w2", [V, J], f32, kind="ExternalInput")
    out = nc.dram_tensor("out", [T, U, V], f32, kind="ExternalOutput")
    with tile.TileContext(nc) as tc:
        _emit(nc, tc, ft, gu, w1, w2, out)
    nc.compile()
    _NC_CACHE = nc
    return nc


def run(ft, gu, w1, w2, trace=False):
    """Run the SPMD kernel; returns (output [B,T,U,V], BassKernelResults)."""
    nc = _build()
    w1c = np.ascontiguousarray(w1, dtype=np.float32)
    w2c = np.ascontiguousarray(w2, dtype=np.float32)
    in_maps = [
        {
            "ft": np.ascontiguousarray(ft[b], dtype=np.float32),
            "gu": np.ascontiguousarray(gu[b], dtype=np.float32),
            "w1": w1c,
            "w2": w2c,
        }
        for b in range(B)
    ]
    res = run_bass_kernel_spmd(
        nc, in_maps, core_ids=list(range(N_CORES)), trace=trace
    )
    out = np.stack([res.results[c]["out"] for c in range(N_CORES)], axis=0)
    return out, res


def kernel(ft, gu, w1, w2):
    out, _ = run(ft, gu, w1, w2, trace=False)
    return out
